# revision 1
# baseline (speedup 1.0000x reference)
"""Trainium2 Bass kernel for AnatomicalMaskedLinear (block-masked dense layer).

Reference op:
    mask  = kron(adjacency, ones(256, 128))            # (21*256, 21*128)
    y     = x.reshape(B, 21*128) @ (weight*mask).T + bias
    out   = y.reshape(B, 21, 256)

Strategy:
  * The mask zeroes whole (256 out x 128 in) blocks; blocks with A[i,j]==0
    contribute nothing, so only nonzero blocks are shipped/matmul'd.
  * 8 cores = 4 batch quarters x 2 node-row halves. Each node's 256 output
    rows share one adjacency row, so the two 128-row halves of every node
    block have identical sparsity structure -> all 8 cores run the SAME
    instruction schedule (one SPMD graph), only the data differs.
  * Per core: xT (2688 x 1024, shipped fp16), packed nonzero weight blocks
    ([128k x 128o] each, pre-transposed, fp16), bias slice. Matmuls
    accumulate in fp32 PSUM: out[o,b] += sum_k wT[k,o] * xT[k,b]; bias adds
    during the DVE evacuation; writes yT (2688 x 1024 f32). Host reassembles
    the full (4096, 21, 256) f32 output.
  * Nodes are processed in a greedy order that minimizes new x-blocks early.
    x blocks and per-node weight chunks each alternate between the two HWDGE
    queues (Sync/Scalar); output stores ride Scalar only, so no load ever
    queues behind a compute-dependent store. The 6-deep weight pool lets
    weight DMAs issue far enough ahead that the 474-matmul stream runs
    gap-free at the 216ns/matmul fp16 streaming floor (measured 126.6us
    end-to-end at 2.4GHz, rel err 2.5e-4).
"""

import os
import numpy as np

NUM_NODES = 21
IN_F = 128
OUT_F = 256
BATCH = 4096
N_CORES = 8
P_BATCH = 4                      # batch ways
B_C = BATCH // P_BATCH           # 1024 batch rows per core
B_TILE = 512                     # matmul moving free dim
N_BT = B_C // B_TILE             # 2 batch tiles per core
K_TOTAL = NUM_NODES * IN_F       # 2688
O_C = NUM_NODES * 128            # 2688 out rows per core (half of each node)

_CACHE = {}                      # schedule key -> (nc, sched)


def _node_order(active):
    """Greedy: minimize newly-required x blocks at each step."""
    loaded = set()
    remaining = set(range(NUM_NODES))
    order = []
    while remaining:
        nxt = min(remaining,
                  key=lambda i: (len(set(active[i]) - loaded), len(active[i]), i))
        order.append(nxt)
        loaded |= set(active[nxt])
        remaining.remove(nxt)
    return order


def _build_schedule(adjacency):
    """[(i, [j...], zero_pad)] in greedy node order; >=1 slot per node."""
    A = np.asarray(adjacency) != 0
    active = {i: [int(j) for j in np.where(A[i])[0]] for i in range(NUM_NODES)}
    sched = []
    for i in _node_order(active):
        js = active[i]
        if js:
            sched.append((i, tuple(js), False))
        else:
            sched.append((i, (0,), True))
    return tuple(sched)


def _build_graph(sched):
    import concourse.tile as tile
    from concourse import bacc, mybir

    S = sum(len(js) for _, js, _ in sched)
    max_nnz = max(len(js) for _, js, _ in sched)
    f32 = mybir.dt.float32
    f16 = mybir.dt.float16

    nc = bacc.Bacc("TRN2", target_bir_lowering=False, debug=False,
                   num_devices=N_CORES)

    xt_d = nc.declare_dram_parameter("xt", [K_TOTAL, B_C], f16, isOutput=False)
    wp_d = nc.declare_dram_parameter("wp", [128, S * 128], f16, isOutput=False)
    bias_d = nc.declare_dram_parameter("biasr", [128, NUM_NODES], f32,
                                       isOutput=False)
    out_d = nc.declare_dram_parameter("out", [O_C, B_C], f32, isOutput=True)

    with tile.TileContext(nc) as tc:
        with (
            tc.tile_pool(name="const", bufs=1) as constp,
            tc.tile_pool(name="wbfp", bufs=6) as wbfp,
            tc.tile_pool(name="persist", bufs=1) as persist,
            tc.tile_pool(name="psum", bufs=8, space="PSUM") as psump,
            tc.tile_pool(name="outp", bufs=8) as outp,
        ):
            bias_sb = constp.tile([128, NUM_NODES], f32)
            nc.scalar.dma_start(out=bias_sb[:], in_=bias_d[:])

            xt_bf = persist.tile([128, NUM_NODES * B_C], f16)

            # per-node new-x lists, then emit with prefetch distance 1
            new_js = []
            seen = set()
            for i, js, _zero in sched:
                cur = [j for j in js if j not in seen]
                seen |= set(cur)
                new_js.append(cur)

            def load_x(node_idx):
                for j in new_js[node_idx]:
                    eng = nc.sync if (len(loaded_x) % 2 == 0) else nc.scalar
                    loaded_x.add(j)
                    eng.dma_start(out=xt_bf[:, j * B_C:(j + 1) * B_C],
                                  in_=xt_d[j * 128:(j + 1) * 128, :])

            loaded_x = set()
            load_x(0)
            s0 = 0
            for k, (i, js, _zero) in enumerate(sched):
                nj = len(js)
                wbf = wbfp.tile([128, max_nnz * 128], f16, tag="wbf")
                weng = nc.sync if k % 2 == 0 else nc.scalar
                weng.dma_start(out=wbf[:, :nj * 128],
                               in_=wp_d[:, s0 * 128:(s0 + nj) * 128])
                if k + 1 < len(sched):
                    load_x(k + 1)

                for bt in range(N_BT):
                    ps = psump.tile([128, B_TILE], f32, tag="acc",
                                    name=f"acc_{i}_{bt}")
                    for idx, j in enumerate(js):
                        nc.tensor.matmul(
                            ps[:],
                            wbf[:, idx * 128:(idx + 1) * 128],
                            xt_bf[:, j * B_C + bt * B_TILE:
                                  j * B_C + bt * B_TILE + B_TILE],
                            start=(idx == 0),
                            stop=(idx == nj - 1),
                        )
                    ot = outp.tile([128, B_TILE], f32, tag="ot")
                    nc.vector.tensor_scalar_add(ot[:], ps[:],
                                                bias_sb[:, i:i + 1])
                    nc.scalar.dma_start(
                        out=out_d[i * 128:(i + 1) * 128,
                                  bt * B_TILE:(bt + 1) * B_TILE],
                        in_=ot[:],
                    )
                s0 += nj

    nc.compile()
    return nc


def _get_graph(adjacency):
    sched = _build_schedule(adjacency)
    if sched not in _CACHE:
        _CACHE[sched] = (_build_graph(sched), sched)
    return _CACHE[sched]


def _pack_inputs(x, weight, bias, sched):
    """Build the 8 per-core input maps (host-side slicing/layout only)."""
    x = np.asarray(x, dtype=np.float32).reshape(BATCH, K_TOTAL).astype(np.float16)
    weight = np.asarray(weight, dtype=np.float32).astype(np.float16)
    bias = np.asarray(bias, dtype=np.float32)

    flat = []  # (i, j, zero) in slot order
    for i, js, zero in sched:
        for j in js:
            flat.append((i, j, zero))
    S = len(flat)

    w5 = weight.reshape(NUM_NODES, 2, 128, NUM_NODES, IN_F)  # i, h, o, j, k
    w5t = w5.transpose(1, 4, 0, 3, 2)                        # h, k, i, j, o

    si = np.array([f[0] for f in flat])
    sj = np.array([f[1] for f in flat])
    szero = np.array([f[2] for f in flat])

    wp_h = []
    for h in range(2):
        wp = np.ascontiguousarray(w5t[h][:, si, sj, :])      # [128, S, 128]
        if szero.any():
            wp[:, szero, :] = 0.0
        wp_h.append(wp.reshape(128, S * 128))

    bias3 = bias.reshape(NUM_NODES, 2, 128)
    bias_h = [np.ascontiguousarray(bias3[:, h, :].T) for h in range(2)]

    in_maps = []
    for c in range(N_CORES):
        bq, h = divmod(c, 2)
        xt = np.ascontiguousarray(x[bq * B_C:(bq + 1) * B_C].T)  # [2688, 1024]
        in_maps.append({
            "xt": xt,
            "wp": wp_h[h],
            "biasr": bias_h[h],
        })
    return in_maps


def _gather_output(results):
    y = np.empty((P_BATCH, B_C, NUM_NODES, 2, 128), dtype=np.float32)
    for c in range(N_CORES):
        bq, h = divmod(c, 2)
        oc = results[c]["out"].reshape(NUM_NODES, 128, B_C)
        y[bq, :, :, h, :] = oc.transpose(2, 0, 1)
    return y.reshape(BATCH, NUM_NODES, OUT_F)


def _ensure_axon_profile_hook():
    """Provide antenv.axon_hooks if the image lacks it (no-op otherwise).

    concourse.bass_utils imports antenv.axon_hooks on the trace path; some
    images miss the module, which would turn BASS_TRACE=1 into an
    ImportError. Registers the standard ctypes NTFF hook when possible.
    """
    try:
        import antenv.axon_hooks  # noqa: F401
        return
    except ImportError:
        pass
    try:
        import antenv
    except ImportError:
        return
    import contextlib
    import ctypes
    import sys
    import types

    hook = None
    try:
        lib = ctypes.CDLL("/opt/axon/libaxon_pjrt.so")
        if hasattr(lib, "axon_start_nrt_profile"):
            lib.axon_start_nrt_profile.argtypes = [
                ctypes.POINTER(ctypes.c_int64), ctypes.c_size_t]
            lib.axon_start_nrt_profile.restype = ctypes.c_int64
            lib.axon_stop_nrt_profile.argtypes = [ctypes.c_char_p]
            lib.axon_stop_nrt_profile.restype = ctypes.c_int64

            @contextlib.contextmanager
            def hook(output_dir, device_ids):
                import jax
                jax.devices()
                if device_ids:
                    ids = (ctypes.c_int64 * len(device_ids))(*device_ids)
                    rc = lib.axon_start_nrt_profile(ids, len(device_ids))
                else:
                    rc = lib.axon_start_nrt_profile(None, 0)
                if rc != 0:
                    raise RuntimeError(f"axon_start_nrt_profile rc={rc}")
                try:
                    yield
                finally:
                    lib.axon_stop_nrt_profile(str(output_dir).encode())
    except OSError:
        hook = None

    mod = types.ModuleType("antenv.axon_hooks")
    mod._hook = hook
    mod.get_axon_ntff_profile_hook = lambda: mod._hook

    def _set(h):
        mod._hook = h

    mod.set_axon_ntff_profile_hook = _set
    sys.modules["antenv.axon_hooks"] = mod
    antenv.axon_hooks = mod


def kernel(x, weight, bias, adjacency):
    from concourse.bass_utils import run_bass_kernel_spmd

    _ensure_axon_profile_hook()
    nc, sched = _get_graph(adjacency)
    in_maps = _pack_inputs(x, weight, bias, sched)

    kwargs = {}
    if os.environ.get("KERNEL_TRACE"):
        kwargs["trace"] = True
        tcores = os.environ.get("KERNEL_TRACE_CORES")
        if tcores:
            kwargs["trace_cores"] = [int(t) for t in tcores.split(",")]

    res = run_bass_kernel_spmd(nc, in_maps, core_ids=list(range(N_CORES)),
                               **kwargs)
    kernel.last_result = res
    return _gather_output(res.results)


kernel.last_result = None



# revision 2
# speedup vs baseline: 1.0066x; 1.0066x over previous
"""Trainium2 Bass kernel for AnatomicalMaskedLinear (block-masked dense layer).

Reference op:
    mask  = kron(adjacency, ones(256, 128))            # (21*256, 21*128)
    y     = x.reshape(B, 21*128) @ (weight*mask).T + bias
    out   = y.reshape(B, 21, 256)

Strategy (v2):
  * Only nonzero (256o x 128i) blocks are shipped/matmul'd. 8 cores =
    4 batch quarters x 2 node-row halves; all cores run one SPMD graph.
  * Per core the 237-slot fp16 matmul stream (474 MMs of 512 moving cols)
    is the hard floor (~102.4us @2.4GHz). Everything else is arranged to
    keep the PE gap-free from t~2us:
      - whole W (fp16, slot-packed) and x (fp16, phase-major, first-use
        order) live in SBUF; prefix DMAs are demand-ordered and balanced
        across the two HWDGE queues (sync/scalar), stores ride gpsimd
        (SWDGE) early so no load queues behind a compute-dependent store.
      - batch is processed in 2 phases of 512 cols so only half of x
        gates the stream prefix.
      - node order is hill-climbed against an analytic DMA-stall bound.
      - 8 garbage warm-up matmuls ramp the PE clock (HAM) to 8/8 while
        the first DMAs are in flight.
      - the last node's evacuation is split in two 256-col halves stored
        on the two idle HWDGE queues to shorten the drain tail.
"""

import os
import numpy as np

NUM_NODES = 21
IN_F = 128
OUT_F = 256
BATCH = 4096
N_CORES = 8
P_BATCH = 4                      # batch ways
B_C = BATCH // P_BATCH           # 1024 batch rows per core
B_TILE = 512                     # matmul moving free dim (one phase)
N_PH = 2                         # batch phases per core
K_TOTAL = NUM_NODES * IN_F       # 2688
O_C = NUM_NODES * 128            # 2688 out rows per core (half of each node)

_CACHE = {}                      # schedule key -> (nc, sched, xorder)

# analytic model constants for the node-order optimizer
_MM_NS = 216.0                   # per 512-col fp16 matmul, warm
_BW = 358.0                      # bytes/ns aggregate DMA bandwidth per core
_ISSUE = 1500.0                  # ns of DMA issue/latency ramp


def _stall_bound(order, active):
    """Worst (data-ready - mm-schedule) over phase-0/1 checkpoints."""
    xseen = set()
    xb, wb, mmper = [], [], []
    cx = cw = 0
    for i in order:
        js = active[i]
        new = [j for j in js if j not in xseen]
        xseen |= set(new)
        cx += len(new) * 128 * B_TILE * 2
        cw += max(len(js), 1) * 128 * 128 * 2
        xb.append(cx)
        wb.append(cw)
        mmper.append(max(len(js), 1) * _MM_NS)
    worst = -1e18
    cm = 0.0
    for p in range(N_PH):
        for k in range(len(order)):
            need = xb[-1] * p + xb[k] + (wb[k] if p == 0 else wb[-1])
            stall = _ISSUE + need / _BW - cm
            if stall > worst:
                worst = stall
            cm += mmper[k]
    return worst


def _node_order(active):
    """Greedy seed + deterministic hill-climb on the DMA stall bound."""
    import random
    loaded = set()
    remaining = set(range(NUM_NODES))
    order = []
    while remaining:
        nxt = min(remaining,
                  key=lambda i: (len(set(active[i]) - loaded),
                                 len(active[i]), i))
        order.append(nxt)
        loaded |= set(active[nxt])
        remaining.remove(nxt)
    rnd = random.Random(0)
    cur = list(order)
    curs = _stall_bound(cur, active)
    n = len(cur)
    for _ in range(8000):
        a, b = rnd.sample(range(n), 2)
        cur[a], cur[b] = cur[b], cur[a]
        s = _stall_bound(cur, active)
        if s <= curs:
            curs = s
        else:
            cur[a], cur[b] = cur[b], cur[a]
    return cur


def _build_schedule(adjacency):
    """[(i, [j...], zero_pad)] in optimized node order; >=1 slot per node."""
    A = np.asarray(adjacency) != 0
    active = {i: [int(j) for j in np.where(A[i])[0]] for i in range(NUM_NODES)}
    sched = []
    for i in _node_order(active):
        js = active[i]
        if js:
            sched.append((i, tuple(js), False))
        else:
            sched.append((i, (0,), True))
    return tuple(sched)


def _x_first_use(sched):
    """x blocks in first-use order (only blocks actually used)."""
    xorder = []
    seen = set()
    for _i, js, _z in sched:
        for j in js:
            if j not in seen:
                seen.add(j)
                xorder.append(j)
    return xorder


def _build_graph(sched):
    import concourse.tile as tile
    from concourse import bacc, mybir

    xorder = _x_first_use(sched)
    xpos = {j: s for s, j in enumerate(xorder)}
    NX = len(xorder)
    S = sum(len(js) for _, js, _ in sched)
    f32 = mybir.dt.float32
    f16 = mybir.dt.float16

    nc = bacc.Bacc("TRN2", target_bir_lowering=False, debug=False,
                   num_devices=N_CORES)

    xt_d = nc.declare_dram_parameter("xt", [128, N_PH * NX * B_TILE], f16,
                                     isOutput=False)
    wp_d = nc.declare_dram_parameter("wp", [128, S * 128], f16, isOutput=False)
    bias_d = nc.declare_dram_parameter("biasr", [128, NUM_NODES], f32,
                                       isOutput=False)
    out_d = nc.declare_dram_parameter("out", [O_C, B_C], f32, isOutput=True)

    # ---- DMA plan: demand-ordered prefix, balanced across the 2 HWDGE qs
    items = []          # ("w"|"x0", a, b) -> slot or x-s ranges
    slot0 = []
    s = 0
    xdone = 0
    xneed = []          # cumulative first-use count per node
    seen = set()
    for k, (i, js, _z) in enumerate(sched):
        slot0.append(s)
        items.append(("w", s, s + len(js)))
        s += len(js)
        new = [j for j in js if j not in seen]
        seen |= set(new)
        end = xdone + len(new)
        csz = 2 if k == 0 else 3
        while xdone < end:
            e = min(xdone + csz, end)
            items.append(("x0", xdone, e))
            xdone = e
    qb = [0, 0]
    qitems = [[], []]
    for it in items:
        kind, a, b = it
        nbytes = (b - a) * 128 * ((128 if kind == "w" else B_TILE) * 2)
        qi = 0 if qb[0] <= qb[1] else 1
        qitems[qi].append(it)
        qb[qi] += nbytes

    last_k = len(sched) - 1

    with tile.TileContext(nc) as tc:
        with (
            tc.tile_pool(name="persist", bufs=1) as persist,
            tc.tile_pool(name="psum", bufs=8, space="PSUM") as psump,
            tc.tile_pool(name="outp", bufs=8) as outp,
            tc.tile_pool(name="tailp", bufs=1) as tailp,
        ):
            warm = persist.tile([128, B_TILE], f16, tag="warm")
            nc.gpsimd.memset(warm[:], 0.0)
            bias_sb = persist.tile([128, NUM_NODES], f32, tag="bias")
            nc.gpsimd.dma_start(out=bias_sb[:], in_=bias_d[:])

            xt = persist.tile([128, N_PH * NX * B_TILE], f16, tag="xt")
            w_all = persist.tile([128, S * 128], f16, tag="wall")

            for qi, eng in ((0, nc.sync), (1, nc.scalar)):
                for kind, a, b in qitems[qi]:
                    if kind == "w":
                        eng.dma_start(out=w_all[:, a * 128:b * 128],
                                      in_=wp_d[:, a * 128:b * 128])
                    else:
                        eng.dma_start(out=xt[:, a * B_TILE:b * B_TILE],
                                      in_=xt_d[:, a * B_TILE:b * B_TILE])
            # phase-1 x rides sync behind the prefix
            for a in range(0, NX, 4):
                b = min(a + 4, NX)
                nc.sync.dma_start(
                    out=xt[:, (NX + a) * B_TILE:(NX + b) * B_TILE],
                    in_=xt_d[:, (NX + a) * B_TILE:(NX + b) * B_TILE])

            # PE clock warm-up on garbage zeros
            for wi in range(8):
                wps = psump.tile([128, B_TILE], f32, tag="acc",
                                 name=f"warm_{wi}")
                nc.tensor.matmul(wps[:], warm[:, :128], warm[:],
                                 start=True, stop=True)

            for h in range(N_PH):
                for k, (i, js, _z) in enumerate(sched):
                    nj = len(js)
                    ps = psump.tile([128, B_TILE], f32, tag="acc",
                                    name=f"acc_{h}_{k}")
                    for idx, j in enumerate(js):
                        st = slot0[k] + idx
                        xc = (h * NX + xpos[j]) * B_TILE
                        nc.tensor.matmul(
                            ps[:],
                            w_all[:, st * 128:(st + 1) * 128],
                            xt[:, xc:xc + B_TILE],
                            start=(idx == 0),
                            stop=(idx == nj - 1),
                        )
                    ob = h * B_TILE
                    if h == 1 and k == last_k:
                        # split the final drain across the idle HWDGE queues
                        o1 = tailp.tile([128, 256], f32, tag="oth1")
                        o2 = tailp.tile([128, 256], f32, tag="oth2")
                        nc.vector.tensor_scalar_add(o1[:], ps[:, 0:256],
                                                    bias_sb[:, i:i + 1])
                        nc.vector.tensor_scalar_add(o2[:], ps[:, 256:512],
                                                    bias_sb[:, i:i + 1])
                        nc.scalar.dma_start(
                            out=out_d[i * 128:(i + 1) * 128, ob:ob + 256],
                            in_=o1[:])
                        nc.sync.dma_start(
                            out=out_d[i * 128:(i + 1) * 128,
                                      ob + 256:ob + 512],
                            in_=o2[:])
                    else:
                        ot = outp.tile([128, B_TILE], f32, tag="ot")
                        nc.vector.tensor_scalar_add(ot[:], ps[:],
                                                    bias_sb[:, i:i + 1])
                        eng = nc.gpsimd if (h == 0 and k < 13) else nc.scalar
                        eng.dma_start(
                            out=out_d[i * 128:(i + 1) * 128, ob:ob + B_TILE],
                            in_=ot[:])

    nc.compile()
    return nc


def _get_graph(adjacency):
    sched = _build_schedule(adjacency)
    if sched not in _CACHE:
        _CACHE[sched] = (_build_graph(sched), sched)
    return _CACHE[sched]


def _pack_inputs(x, weight, bias, sched):
    """Build the 8 per-core input maps (host-side slicing/layout only)."""
    xorder = _x_first_use(sched)
    NX = len(xorder)
    x = np.asarray(x, dtype=np.float32).reshape(BATCH, NUM_NODES, IN_F)
    x16 = x.astype(np.float16)
    weight = np.asarray(weight, dtype=np.float32).astype(np.float16)
    bias = np.asarray(bias, dtype=np.float32)

    flat = []  # (i, j, zero) in slot order
    for i, js, zero in sched:
        for j in js:
            flat.append((i, j, zero))
    S = len(flat)

    w5 = weight.reshape(NUM_NODES, 2, 128, NUM_NODES, IN_F)  # i, h, o, j, k
    w5t = w5.transpose(1, 4, 0, 3, 2)                        # h, k, i, j, o

    si = np.array([f[0] for f in flat])
    sj = np.array([f[1] for f in flat])
    szero = np.array([f[2] for f in flat])

    wp_h = []
    for h in range(2):
        wp = np.ascontiguousarray(w5t[h][:, si, sj, :])      # [128, S, 128]
        if szero.any():
            wp[:, szero, :] = 0.0
        wp_h.append(wp.reshape(128, S * 128))

    bias3 = bias.reshape(NUM_NODES, 2, 128)
    bias_h = [np.ascontiguousarray(bias3[:, h, :].T) for h in range(2)]

    xt_q = []
    for bq in range(P_BATCH):
        xc = x16[bq * B_C:(bq + 1) * B_C]                    # [1024, 21, 128]
        xc4 = xc.reshape(N_PH, B_TILE, NUM_NODES, IN_F)      # ph, b, j, p
        xr = xc4[:, :, xorder, :]                            # ph, b, s, p
        xt = np.ascontiguousarray(xr.transpose(3, 0, 2, 1))  # p, ph, s, b
        xt_q.append(xt.reshape(128, N_PH * NX * B_TILE))

    in_maps = []
    for c in range(N_CORES):
        bq, h = divmod(c, 2)
        in_maps.append({
            "xt": xt_q[bq],
            "wp": wp_h[h],
            "biasr": bias_h[h],
        })
    return in_maps


def _gather_output(results):
    y = np.empty((P_BATCH, B_C, NUM_NODES, 2, 128), dtype=np.float32)
    for c in range(N_CORES):
        bq, h = divmod(c, 2)
        oc = results[c]["out"].reshape(NUM_NODES, 128, B_C)
        y[bq, :, :, h, :] = oc.transpose(2, 0, 1)
    return y.reshape(BATCH, NUM_NODES, OUT_F)


def _ensure_axon_profile_hook():
    """Provide antenv.axon_hooks if the image lacks it (no-op otherwise).

    concourse.bass_utils imports antenv.axon_hooks on the trace path; some
    images miss the module, which would turn BASS_TRACE=1 into an
    ImportError. Registers the standard ctypes NTFF hook when possible.
    """
    try:
        import antenv.axon_hooks  # noqa: F401
        return
    except ImportError:
        pass
    try:
        import antenv
    except ImportError:
        return
    import contextlib
    import ctypes
    import sys
    import types

    hook = None
    try:
        lib = ctypes.CDLL("/opt/axon/libaxon_pjrt.so")
        if hasattr(lib, "axon_start_nrt_profile"):
            lib.axon_start_nrt_profile.argtypes = [
                ctypes.POINTER(ctypes.c_int64), ctypes.c_size_t]
            lib.axon_start_nrt_profile.restype = ctypes.c_int64
            lib.axon_stop_nrt_profile.argtypes = [ctypes.c_char_p]
            lib.axon_stop_nrt_profile.restype = ctypes.c_int64

            @contextlib.contextmanager
            def hook(output_dir, device_ids):
                import jax
                jax.devices()
                if device_ids:
                    ids = (ctypes.c_int64 * len(device_ids))(*device_ids)
                    rc = lib.axon_start_nrt_profile(ids, len(device_ids))
                else:
                    rc = lib.axon_start_nrt_profile(None, 0)
                if rc != 0:
                    raise RuntimeError(f"axon_start_nrt_profile rc={rc}")
                try:
                    yield
                finally:
                    lib.axon_stop_nrt_profile(str(output_dir).encode())
    except OSError:
        hook = None

    mod = types.ModuleType("antenv.axon_hooks")
    mod._hook = hook
    mod.get_axon_ntff_profile_hook = lambda: mod._hook

    def _set(h):
        mod._hook = h

    mod.set_axon_ntff_profile_hook = _set
    sys.modules["antenv.axon_hooks"] = mod
    antenv.axon_hooks = mod


def kernel(x, weight, bias, adjacency):
    from concourse.bass_utils import run_bass_kernel_spmd

    _ensure_axon_profile_hook()
    nc, sched = _get_graph(adjacency)
    in_maps = _pack_inputs(x, weight, bias, sched)

    kwargs = {}
    if os.environ.get("KERNEL_TRACE"):
        kwargs["trace"] = True
        tcores = os.environ.get("KERNEL_TRACE_CORES")
        if tcores:
            kwargs["trace_cores"] = [int(t) for t in tcores.split(",")]

    res = run_bass_kernel_spmd(nc, in_maps, core_ids=list(range(N_CORES)),
                               **kwargs)
    kernel.last_result = res
    return _gather_output(res.results)


kernel.last_result = None
